# revision 20
# baseline (speedup 1.0000x reference)
"""Trainium2 Bass kernel for nn_CapsuleLinear (k-means 'dot' routing, 3 iters).

Math (per example b):
  priors[o,i,v] = sum_l W[o,i,v,l] * x[b,i,l]
  out0 = mean_i priors
  3x: n = normalize(out); logits[o,i] = sum_v priors*n; probs = softmax_o(logits);
      out[o,v] = sum_i probs*priors
  result = squash(out) + bias

Sharding: data-parallel over batch B=64 across 8 cores (8 examples/core).

Per-core layout (P = 128 partitions = (i_p in 0..15, b in 0..7), p = i_p*8+b):
  priors SBUF bf16 [128, ib=32, v=16, o=64], full i = ib*16 + i_p.
  Priors are produced by PE matmuls: for each ib, lhsT = block-diagonal x
  ([128=(i_sub,l), 128=(i_sub,b)]) and rhs = W2[ib] ([128=(i_sub,l), 1024=(o,v)]),
  giving PSUM [(i_sub,b), (o,v)].
  out0 comes from a second matmul with lhsT = x replicated over i_p' columns
  ([128=(i_sub,l), 128=(i_p',b)]), accumulated over all ib in PSUM -> the
  [128, (o,v)] result is replicated over i_p, so no partition broadcast is
  ever needed for the routing state.
  Routing iterations run on DVE (bf16 2x ops) + ACT (exp); the partition-dim
  part of the i-reduction runs on PE via a fixed 0/1 matrix (ones where
  p%8 == m%8), which also re-broadcasts the new `out` over all partitions.
"""

import numpy as np

import concourse.bass as bass
import concourse.bacc as bacc
import concourse.tile as tile
from concourse import mybir
from concourse.bass_utils import run_bass_kernel_spmd

B, I, O, V, L = 64, 512, 64, 16, 8
NCORES = 8
BL = B // NCORES  # 8 examples per core
IB = I // 16  # 32 blocks of 16 i's
IP = 16  # i_p values per partition group

f32 = mybir.dt.float32
f32r = mybir.dt.float32r
bf16 = mybir.dt.bfloat16

LAST_RESULT = None  # stash of BassKernelResults for test harness


def _build_kernel():
    nc = bacc.Bacc(
        "TRN2",
        target_bir_lowering=False,
        debug=False,
        enable_asserts=False,
        num_devices=NCORES,
    )
    w2_d = nc.dram_tensor("w2", [IB, 128, O * V], bf16, kind="ExternalInput")
    xdg_d = nc.dram_tensor("xdg", [IB, 128, 128], bf16, kind="ExternalInput")
    ones_d = nc.dram_tensor("onesd", [128, 128], bf16, kind="ExternalInput")
    bias_d = nc.dram_tensor("biasT", [V, O], f32, kind="ExternalInput")
    out_d = nc.dram_tensor("out", [BL, V, O], f32, kind="ExternalOutput")

    with tile.TileContext(nc) as tc:
        _body(nc, tc, w2_d, xdg_d, ones_d, bias_d, out_d)
    nc.compile()
    return nc


def _body(nc, tc, w2_d, xdg_d, ones_d, bias_d, out_d):
    AL = mybir.AluOpType
    X = mybir.AxisListType.X
    AF = mybir.ActivationFunctionType

    from contextlib import ExitStack

    with ExitStack() as ctx:
        big = ctx.enter_context(tc.tile_pool(name="big", bufs=1))
        wp = ctx.enter_context(tc.tile_pool(name="wp", bufs=6))
        xp = ctx.enter_context(tc.tile_pool(name="xp", bufs=4))
        sm = ctx.enter_context(tc.tile_pool(name="sm", bufs=1))
        pr_ps = ctx.enter_context(tc.tile_pool(name="prps", bufs=3, space="PSUM"))
        out_ps = ctx.enter_context(tc.tile_pool(name="outps", bufs=2, space="PSUM"))

        # ---- persistent tiles ----
        priors = big.tile([128, IB, V, O], bf16)
        prod = big.tile([128, IB, V, O], bf16)
        logits = big.tile([128, IB, O], f32)
        probs = big.tile([128, IB, O], bf16)
        ones_t = big.tile([128, 128], bf16)
        bias_t = big.tile([BL, V, O], f32)

        onesf = big.tile([128, 128], f32)
        nc.sync.dma_start(out=ones_t[:], in_=ones_d[:])
        nc.vector.tensor_copy(out=onesf[:], in_=ones_t[:])
        nc.sync.dma_start(
            out=bias_t[:], in_=bias_d[:].unsqueeze(0).broadcast_to([BL, V, O])
        )

        # ---- phase 1: priors + out0 ----
        out0 = out_ps.tile([128, O, V], f32, bufs=1)  # (o, v) order, replicated
        out0f = out0[:].rearrange("p o v -> p (o v)")
        acc = sm.tile([128, O * V], f32, tag="acc")
        for ib in range(IB):
            w = wp.tile([128, O * V], bf16, tag="w")
            nc.sync.dma_start(out=w[:], in_=w2_d[ib])
            xd = xp.tile([128, 128], bf16, tag="xd")
            nc.sync.dma_start(out=xd[:], in_=xdg_d[ib])
            pp = pr_ps.tile([128, O * V], f32, tag="pp")
            for h in range(2):
                sl = slice(h * 512, (h + 1) * 512)
                nc.tensor.matmul(pp[:, sl], xd[:], w[:, sl], start=True, stop=True)
            # accumulate sum over ib on the (otherwise idle) DVE
            if ib == 0:
                nc.vector.tensor_scalar_add(acc[:], pp[:], 0.0)
            else:
                nc.vector.tensor_add(acc[:], acc[:], pp[:])
            # PSUM (o,v) -> SBUF priors[:, ib] in (v, o) order, cast to bf16
            # (on ACT so the DVE stays free for routing)
            ppv = pp[:].rearrange("p (o v) -> p v o", o=O)
            nc.scalar.copy(out=priors[:, ib], in_=ppv)

        # finish out0 = sum over i_p of acc (ones-matmul also re-broadcasts)
        for h in range(2):
            sl = slice(h * 512, (h + 1) * 512)
            nc.tensor.matmul(
                out0f[:, sl],
                onesf[:],
                acc[:, sl],
                start=True,
                stop=True,
                skip_group_check=True,
            )

        # ---- phase 2: routing iterations ----
        out_prev = out0
        prev_vo = False  # out0 is in (o, v) order; updates are (v, o)
        for t in range(3):
            if prev_vo:
                src_ov = out_prev[:].transpose([0, 2, 1])  # [128, O, V] view
                src_vo = out_prev[:]
            else:
                src_ov = out_prev[:]
                src_vo = out_prev[:].transpose([0, 2, 1])
            # n = out / max(||out||, eps)   (scale of out doesn't matter)
            sq = sm.tile([128, O, V], f32, tag="sq")
            nc.scalar.square(sq[:], src_ov)
            nsq = sm.tile([128, O], f32, tag="nsq")
            nc.vector.tensor_reduce(out=nsq[:], in_=sq[:], axis=X, op=AL.add)
            # rn = 1/sqrt(nsq) = exp(-0.5*ln(nsq)); ln/exp/square/copy share
            # one ACT table set, so no table reloads anywhere in routing
            lnn = sm.tile([128, O], f32, tag="lnn")
            nc.scalar.activation(out=lnn[:], in_=nsq[:], func=AF.Ln)
            rn = sm.tile([128, O], f32, tag="rn")
            nc.scalar.activation(out=rn[:], in_=lnn[:], func=AF.Exp, scale=-0.5)
            ntile = sm.tile([128, V, O], bf16, tag="ntile")
            nc.vector.tensor_mul(
                ntile[:], src_vo, rn[:].unsqueeze(1).broadcast_to([128, V, O])
            )

            # logits[p, ib, o] = sum_v priors * n
            for c in range(8):
                s = slice(c * 4, (c + 1) * 4)
                nc.vector.tensor_mul(
                    prod[:, s],
                    priors[:, s],
                    ntile[:].unsqueeze(1).broadcast_to([128, 4, V, O]),
                )
            nc.vector.tensor_add(prod[:, :, 0:8], prod[:, :, 0:8], prod[:, :, 8:16])
            nc.vector.tensor_add(prod[:, :, 0:4], prod[:, :, 0:4], prod[:, :, 4:8])
            nc.vector.tensor_add(prod[:, :, 0:2], prod[:, :, 0:2], prod[:, :, 2:4])
            nc.vector.tensor_add(logits[:], prod[:, :, 0], prod[:, :, 1])

            # probs = softmax over o (no max subtraction; |logits| <~ 4)
            elog = sm.tile([128, IB, O], f32, tag="elog")
            nc.scalar.activation(out=elog[:], in_=logits[:], func=AF.Exp)
            zs = sm.tile([128, IB], f32, tag="zs")
            nc.vector.tensor_reduce(out=zs[:], in_=elog[:], axis=X, op=AL.add)
            rz = sm.tile([128, IB], f32, tag="rz")
            nc.vector.reciprocal(rz[:], zs[:])
            nc.vector.tensor_mul(
                probs[:], elog[:], rz[:].unsqueeze(2).broadcast_to([128, IB, O])
            )

            # out_new[p, v, o] = sum_i probs * priors
            for c in range(16):
                s = slice(c * 2, (c + 1) * 2)
                nc.vector.tensor_mul(
                    prod[:, s],
                    priors[:, s],
                    probs[:, s].unsqueeze(2).broadcast_to([128, 2, V, O]),
                )
            # i-reduction on the PE: accumulate sum over (i_p, ib) of prod2
            # into PSUM via the block-diag ones matrix (also re-broadcasts
            # the result over all partitions).
            out_new = pr_ps.tile([128, V, O], f32, tag="pp")
            onf = out_new[:].rearrange("p v o -> p (v o)")
            for ib in range(IB):
                pslc = prod[:, ib].rearrange("p v o -> p (v o)")
                for h in range(2):
                    sl = slice(h * 512, (h + 1) * 512)
                    nc.tensor.matmul(
                        onf[:, sl],
                        ones_t[:],
                        pslc[:, sl],
                        start=(ib == 0),
                        stop=(ib == IB - 1),
                        skip_group_check=True,
                    )
            out_prev = out_new
            prev_vo = True

        # ---- squash + bias on partitions 0..7 (b rows) ----
        sq2 = sm.tile([128, O, V], f32, tag="sq")
        src_ov = out_prev[:].transpose([0, 2, 1])
        nc.scalar.square(sq2[:], src_ov)
        nsq2 = sm.tile([128, O], f32, tag="nsq")
        nc.vector.tensor_reduce(out=nsq2[:], in_=sq2[:], axis=X, op=AL.add)
        lnn2 = sm.tile([128, O], f32, tag="lnn")
        nc.scalar.activation(out=lnn2[:], in_=nsq2[:], func=AF.Ln)
        norm2 = sm.tile([128, O], f32, tag="norm")
        nc.scalar.activation(out=norm2[:], in_=lnn2[:], func=AF.Exp, scale=0.5)
        den = sm.tile([128, O], f32, tag="den")
        nc.vector.tensor_scalar_add(den[:], nsq2[:], 1.0)
        rden = sm.tile([128, O], f32, tag="rden")
        nc.vector.reciprocal(rden[:], den[:])
        scl = sm.tile([128, O], f32, tag="scl")
        nc.vector.tensor_mul(scl[:], norm2[:], rden[:])

        outf = sm.tile([BL, V, O], f32, tag="outf")
        nc.vector.tensor_mul(
            outf[:],
            out_prev[0:BL],
            scl[0:BL].unsqueeze(1).broadcast_to([BL, V, O]),
        )
        nc.vector.tensor_add(outf[:], outf[:], bias_t[:])
        nc.sync.dma_start(out=out_d[:], in_=outf[:])


_NC_CACHE = []


def _get_nc():
    if not _NC_CACHE:
        _NC_CACHE.append(_build_kernel())
    return _NC_CACHE[0]


def kernel(x, weight, bias):
    global LAST_RESULT
    x = np.asarray(x, dtype=np.float32)
    weight = np.asarray(weight, dtype=np.float32)
    bias = np.asarray(bias, dtype=np.float32)

    import ml_dtypes

    bf = ml_dtypes.bfloat16
    # W2[ib, (i_sub, l), (o, v)] = W[o, ib*16+i_sub, v, l]
    w2 = (
        np.ascontiguousarray(weight.transpose(1, 3, 0, 2))
        .reshape(IB, 128, O * V)
        .astype(bf)
    )
    biasT = np.ascontiguousarray(bias.T)  # [V, O]

    idx = np.arange(128)
    onesd = (idx[:, None] % BL == idx[None, :] % BL).astype(bf)

    in_maps = []
    for c in range(NCORES):
        xc = x[c * BL : (c + 1) * BL]  # [BL, I, L]
        xt = np.ascontiguousarray(xc.transpose(1, 2, 0))  # [I, L, BL] = (i, l, b)
        xt4 = xt.reshape(IB, 16, L, BL)
        xdg = np.zeros((IB, 128, 128), dtype=bf)
        for s in range(16):
            xdg[:, s * L : (s + 1) * L, s * BL : (s + 1) * BL] = xt4[:, s].astype(bf)
        in_maps.append({"w2": w2, "xdg": xdg, "onesd": onesd, "biasT": biasT})

    nc = _get_nc()
    res = run_bass_kernel_spmd(nc, in_maps, core_ids=list(range(NCORES)))
    LAST_RESULT = res

    outs = []
    for r in res.results:
        o = r["out"]  # [BL, V, O]
        outs.append(np.ascontiguousarray(o.transpose(0, 2, 1)))  # [BL, O, V]
    return np.concatenate(outs, axis=0).astype(np.float32)


if __name__ == "__main__":
    rng = np.random.default_rng(0)
    x = rng.standard_normal((B, I, L), dtype=np.float32)
    w = rng.standard_normal((O, I, V, L), dtype=np.float32) * 0.1
    b = rng.standard_normal((O, V), dtype=np.float32) * 0.1
    out = kernel(x, w, b)
    print("out shape", out.shape, out.dtype)


# revision 25
# speedup vs baseline: 1.2052x; 1.2052x over previous
"""Trainium2 Bass kernel for nn_CapsuleLinear (k-means 'dot' routing, 3 iters).

Math (per example b):
  priors[o,i,v] = sum_l W[o,i,v,l] * x[b,i,l]
  out0 = mean_i priors
  3x: n = normalize(out); logits[o,i] = sum_v priors*n; probs = softmax_o(logits);
      out[o,v] = sum_i probs*priors
  result = squash(out) + bias

Sharding: data-parallel over batch B=64 across 8 cores (8 examples/core).

Per-core layout (P = 128 partitions = (i_p in 0..15, b in 0..7), p = i_p*8+b):
  priors SBUF bf16 [128, ib=32, v=16, o=64], full i = ib*16 + i_p.
  Priors are produced by PE matmuls: for each ib, lhsT = block-diagonal x
  ([128=(i_sub,l), 128=(i_sub,b)]) and rhs = W2[ib] ([128=(i_sub,l), 1024=(o,v)]),
  giving PSUM [(i_sub,b), (o,v)].
  out0 comes from a second matmul with lhsT = x replicated over i_p' columns
  ([128=(i_sub,l), 128=(i_p',b)]), accumulated over all ib in PSUM -> the
  [128, (o,v)] result is replicated over i_p, so no partition broadcast is
  ever needed for the routing state.
  Routing iterations run on DVE (bf16 2x ops) + ACT (exp); the partition-dim
  part of the i-reduction runs on PE via a fixed 0/1 matrix (ones where
  p%8 == m%8), which also re-broadcasts the new `out` over all partitions.
"""

import numpy as np

import concourse.bass as bass
import concourse.bacc as bacc
import concourse.tile as tile
from concourse import mybir
from concourse.bass_utils import run_bass_kernel_spmd

B, I, O, V, L = 64, 512, 64, 16, 8
NCORES = 8
BL = B // NCORES  # 8 examples per core
IB = I // 16  # 32 blocks of 16 i's
IP = 16  # i_p values per partition group

f32 = mybir.dt.float32
f32r = mybir.dt.float32r
bf16 = mybir.dt.bfloat16

LAST_RESULT = None  # stash of BassKernelResults for test harness


def _build_kernel():
    nc = bacc.Bacc(
        "TRN2",
        target_bir_lowering=False,
        debug=False,
        enable_asserts=False,
        num_devices=NCORES,
    )
    w2_d = nc.dram_tensor("w2", [IB, 128, O * V], bf16, kind="ExternalInput")
    xdg_d = nc.dram_tensor("xdg", [IB, 128, 128], bf16, kind="ExternalInput")
    ones_d = nc.dram_tensor("onesd", [128, 128], bf16, kind="ExternalInput")
    bias_d = nc.dram_tensor("biasT", [V, O], f32, kind="ExternalInput")
    out_d = nc.dram_tensor("out", [BL, V, O], f32, kind="ExternalOutput")

    with tile.TileContext(nc) as tc:
        _body(nc, tc, w2_d, xdg_d, ones_d, bias_d, out_d)
    nc.compile()
    return nc


def _body(nc, tc, w2_d, xdg_d, ones_d, bias_d, out_d):
    AL = mybir.AluOpType
    X = mybir.AxisListType.X
    AF = mybir.ActivationFunctionType

    from contextlib import ExitStack

    with ExitStack() as ctx:
        big = ctx.enter_context(tc.tile_pool(name="big", bufs=1))
        wp = ctx.enter_context(tc.tile_pool(name="wp", bufs=6))
        xp = ctx.enter_context(tc.tile_pool(name="xp", bufs=4))
        sm = ctx.enter_context(tc.tile_pool(name="sm", bufs=1))
        pr_ps = ctx.enter_context(tc.tile_pool(name="prps", bufs=3, space="PSUM"))
        out_ps = ctx.enter_context(tc.tile_pool(name="outps", bufs=2, space="PSUM"))

        # ---- persistent tiles ----
        priors = big.tile([128, IB, V, O], bf16)
        prod = big.tile([128, IB, V, O], bf16)
        logits = big.tile([128, IB, O], f32)
        probs = big.tile([128, IB, O], bf16)
        ones_t = big.tile([128, 128], bf16)
        bias_t = big.tile([BL, V, O], f32)

        onesf = big.tile([128, 128], f32)
        nc.sync.dma_start(out=ones_t[:], in_=ones_d[:])
        nc.vector.tensor_copy(out=onesf[:], in_=ones_t[:])
        nc.sync.dma_start(
            out=bias_t[:], in_=bias_d[:].unsqueeze(0).broadcast_to([BL, V, O])
        )

        # ---- phase 1: priors + out0 ----
        out0 = out_ps.tile([128, O, V], f32, bufs=1)  # (o, v) order, replicated
        out0f = out0[:].rearrange("p o v -> p (o v)")
        acc = sm.tile([128, O * V], f32, tag="acc")
        for ib in range(IB):
            w = wp.tile([128, O * V], bf16, tag="w")
            nc.sync.dma_start(out=w[:], in_=w2_d[ib])
            xd = xp.tile([128, 128], bf16, tag="xd")
            nc.sync.dma_start(out=xd[:], in_=xdg_d[ib])
            pp = pr_ps.tile([128, O * V], f32, tag="pp")
            for h in range(2):
                sl = slice(h * 512, (h + 1) * 512)
                nc.tensor.matmul(pp[:, sl], xd[:], w[:, sl], start=True, stop=True)
            # accumulate sum over ib on the (otherwise idle) DVE
            if ib == 0:
                nc.vector.tensor_scalar_add(acc[:], pp[:], 0.0)
            else:
                nc.vector.tensor_add(acc[:], acc[:], pp[:])
            # PSUM (o,v) -> SBUF priors[:, ib] in (v, o) order, cast to bf16
            # (on ACT so the DVE stays free for routing)
            ppv = pp[:].rearrange("p (o v) -> p v o", o=O)
            nc.scalar.copy(out=priors[:, ib], in_=ppv)

        # finish out0 = sum over i_p of acc (ones-matmul also re-broadcasts)
        for h in range(2):
            sl = slice(h * 512, (h + 1) * 512)
            nc.tensor.matmul(
                out0f[:, sl],
                onesf[:],
                acc[:, sl],
                start=True,
                stop=True,
                skip_group_check=True,
            )

        # ---- phase 2: routing iterations ----
        out_prev = out0
        prev_vo = False  # out0 is in (o, v) order; updates are (v, o)
        for t in range(3):
            if prev_vo:
                src_ov = out_prev[:].transpose([0, 2, 1])  # [128, O, V] view
                src_vo = out_prev[:]
            else:
                src_ov = out_prev[:]
                src_vo = out_prev[:].transpose([0, 2, 1])
            # n = out / max(||out||, eps)   (scale of out doesn't matter)
            sq = sm.tile([128, O, V], f32, tag="sq")
            nc.scalar.square(sq[:], src_ov)
            nsq = sm.tile([128, O], f32, tag="nsq")
            nc.vector.tensor_reduce(out=nsq[:], in_=sq[:], axis=X, op=AL.add)
            norm = sm.tile([128, O], f32, tag="norm")
            nc.scalar.sqrt(norm[:], nsq[:])
            rn = sm.tile([128, O], f32, tag="rn")
            nc.vector.reciprocal(rn[:], norm[:])
            ntile = sm.tile([128, V, O], bf16, tag="ntile")
            nc.vector.tensor_mul(
                ntile[:], src_vo, rn[:].unsqueeze(1).broadcast_to([128, V, O])
            )

            # logits[p, ib, o] = sum_v priors * n
            for c in range(8):
                s = slice(c * 4, (c + 1) * 4)
                nc.vector.tensor_mul(
                    prod[:, s],
                    priors[:, s],
                    ntile[:].unsqueeze(1).broadcast_to([128, 4, V, O]),
                )
            nc.vector.tensor_add(prod[:, :, 0:8], prod[:, :, 0:8], prod[:, :, 8:16])
            nc.vector.tensor_add(prod[:, :, 0:4], prod[:, :, 0:4], prod[:, :, 4:8])
            nc.vector.tensor_add(prod[:, :, 0:2], prod[:, :, 0:2], prod[:, :, 2:4])
            nc.vector.tensor_add(logits[:], prod[:, :, 0], prod[:, :, 1])

            # probs = softmax over o (no max subtraction; |logits| <~ 4)
            elog = sm.tile([128, IB, O], f32, tag="elog")
            nc.scalar.activation(out=elog[:], in_=logits[:], func=AF.Exp)
            zs = sm.tile([128, IB], f32, tag="zs")
            nc.vector.tensor_reduce(out=zs[:], in_=elog[:], axis=X, op=AL.add)
            rz = sm.tile([128, IB], f32, tag="rz")
            nc.vector.reciprocal(rz[:], zs[:])
            nc.vector.tensor_mul(
                probs[:], elog[:], rz[:].unsqueeze(2).broadcast_to([128, IB, O])
            )

            # out_new[p, v, o] = sum_i probs * priors
            for c in range(16):
                s = slice(c * 2, (c + 1) * 2)
                nc.vector.tensor_mul(
                    prod[:, s],
                    priors[:, s],
                    probs[:, s].unsqueeze(2).broadcast_to([128, 2, V, O]),
                )
            # i-reduction on the PE: accumulate sum over (i_p, ib) of prod2
            # into PSUM via the block-diag ones matrix (also re-broadcasts
            # the result over all partitions).
            out_new = pr_ps.tile([128, V, O], f32, tag="pp")
            onf = out_new[:].rearrange("p v o -> p (v o)")
            for ib in range(IB):
                pslc = prod[:, ib].rearrange("p v o -> p (v o)")
                for h in range(2):
                    sl = slice(h * 512, (h + 1) * 512)
                    nc.tensor.matmul(
                        onf[:, sl],
                        ones_t[:],
                        pslc[:, sl],
                        start=(ib == 0),
                        stop=(ib == IB - 1),
                        skip_group_check=True,
                    )
            out_prev = out_new
            prev_vo = True

        # ---- squash + bias on partitions 0..7 (b rows) ----
        sq2 = sm.tile([128, O, V], f32, tag="sq")
        src_ov = out_prev[:].transpose([0, 2, 1])
        nc.scalar.square(sq2[:], src_ov)
        nsq2 = sm.tile([128, O], f32, tag="nsq")
        nc.vector.tensor_reduce(out=nsq2[:], in_=sq2[:], axis=X, op=AL.add)
        norm2 = sm.tile([128, O], f32, tag="norm")
        nc.scalar.sqrt(norm2[:], nsq2[:])
        den = sm.tile([128, O], f32, tag="den")
        nc.vector.tensor_scalar_add(den[:], nsq2[:], 1.0)
        rden = sm.tile([128, O], f32, tag="rden")
        nc.vector.reciprocal(rden[:], den[:])
        scl = sm.tile([128, O], f32, tag="scl")
        nc.vector.tensor_mul(scl[:], norm2[:], rden[:])

        outf = sm.tile([BL, V, O], f32, tag="outf")
        nc.vector.tensor_mul(
            outf[:],
            out_prev[0:BL],
            scl[0:BL].unsqueeze(1).broadcast_to([BL, V, O]),
        )
        nc.vector.tensor_add(outf[:], outf[:], bias_t[:])
        nc.sync.dma_start(out=out_d[:], in_=outf[:])


_NC_CACHE = []


def _get_nc():
    if not _NC_CACHE:
        _NC_CACHE.append(_build_kernel())
    return _NC_CACHE[0]


def kernel(x, weight, bias):
    global LAST_RESULT
    x = np.asarray(x, dtype=np.float32)
    weight = np.asarray(weight, dtype=np.float32)
    bias = np.asarray(bias, dtype=np.float32)

    import ml_dtypes

    bf = ml_dtypes.bfloat16
    # W2[ib, (i_sub, l), (o, v)] = W[o, ib*16+i_sub, v, l]
    w2 = (
        np.ascontiguousarray(weight.transpose(1, 3, 0, 2))
        .reshape(IB, 128, O * V)
        .astype(bf)
    )
    biasT = np.ascontiguousarray(bias.T)  # [V, O]

    idx = np.arange(128)
    onesd = (idx[:, None] % BL == idx[None, :] % BL).astype(bf)

    in_maps = []
    for c in range(NCORES):
        xc = x[c * BL : (c + 1) * BL]  # [BL, I, L]
        xt = np.ascontiguousarray(xc.transpose(1, 2, 0))  # [I, L, BL] = (i, l, b)
        xt4 = xt.reshape(IB, 16, L, BL)
        xdg = np.zeros((IB, 128, 128), dtype=bf)
        for s in range(16):
            xdg[:, s * L : (s + 1) * L, s * BL : (s + 1) * BL] = xt4[:, s].astype(bf)
        in_maps.append({"w2": w2, "xdg": xdg, "onesd": onesd, "biasT": biasT})

    nc = _get_nc()
    res = run_bass_kernel_spmd(nc, in_maps, core_ids=list(range(NCORES)))
    LAST_RESULT = res

    outs = []
    for r in res.results:
        o = r["out"]  # [BL, V, O]
        outs.append(np.ascontiguousarray(o.transpose(0, 2, 1)))  # [BL, O, V]
    return np.concatenate(outs, axis=0).astype(np.float32)


if __name__ == "__main__":
    rng = np.random.default_rng(0)
    x = rng.standard_normal((B, I, L), dtype=np.float32)
    w = rng.standard_normal((O, I, V, L), dtype=np.float32) * 0.1
    b = rng.standard_normal((O, V), dtype=np.float32) * 0.1
    out = kernel(x, w, b)
    print("out shape", out.shape, out.dtype)
